# revision 18
# baseline (speedup 1.0000x reference)
"""Trainium2 Bass kernel for nn_BSLoss (text-snake style OHEM loss), 8-core
data-parallel.

Strategy
--------
Host shards the batch dim (16 -> 2 per core) and re-lays every tensor out as
[nchunks, 128, C*Fc] fp32 blocks (partition-blocked spatial, channel blocks
adjacent in the free dim) so each chunk is one contiguous DMA and every
channel block of every tensor shares the same [128, Fc] spatial layout.

Device (per core, identical SPMD program):
  - masks: pos = tr*train (+count), neg = (1-tr)*train (+count), w2=(tr+tcl)*pos
  - 2-class CE for tr and tcl heads: ce = relu(d) + ln(1+exp(-|d|)),
    d = (1-2t)*(logit1-logit0)  [== -log_softmax picked]
  - weighted smooth-L1 over the 32 regression channels via two custom DVE
    ops: q = min(|xm-xp|,1)*(2|xm-xp|-min(..,1)) == 2*smooth_l1, then
    accum += sum(w2 * q) with w2 broadcast across channel blocks
  - per-chunk partial sums land in a [128, NCOLS] stats tile; the masked
    negative-CE values (for global topk OHEM) are written back to HBM
Host merges: sums partials, does the exact global top-k over ~2MB of masked
CE values, and finishes the scalar divisions (exactly mirroring reference
semantics, incl. the n_pos==0 fallbacks).
"""

import numpy as np
import ml_dtypes

import concourse.bacc as bacc
import concourse.mybir as mybir
import concourse.dve_ops as dve_ops
from concourse.dve_spec import (
    Spec, Src0, Src1, C0, Zero, One, AluOp, Bin, minn, lower, _has_src1,
)
from concourse.dve_uop import DveOpSpec
from concourse import tile

F32 = mybir.dt.float32
BF16 = mybir.dt.bfloat16
NP_BF16 = ml_dtypes.bfloat16
ALU = mybir.AluOpType
ACT = mybir.ActivationFunctionType

NCORES = 8
B_PER_CORE = 2
# level -> (H, W, nchunks)
LEVELS = [(3, 160, 160, 8), (4, 80, 80, 2), (5, 40, 40, 1)]
KCH = 16          # regression channels per axis
OHEM_RATIO = 3.0

# stats tile column layout: per chunk-slot t, base = t*12
C_NPOS, C_NEGCNT, C_LOSSPOS, C_TCLPOS, C_TCLALL, C_REGX, C_REGY = range(7)
C_DUM0, C_DUM1, C_DUM2, C_DUM3, C_DUM4 = range(7, 12)
COLS_PER_CHUNK = 12
N_CHUNK_SLOTS = sum(nch for _, _, _, nch in LEVELS)
STATS_COLS = COLS_PER_CHUNK * N_CHUNK_SLOTS


def _np_sl1q(d):
    a = np.abs(d)
    m = np.minimum(a, 1.0)
    return m * (a + a - m)   # == 2 * smooth_l1(d)


def _register_custom_ops():
    """Register our fused DVE ops (idempotent)."""
    # QSL1: out = q(Src0 - Src1), q(d) = min(|d|,1)*(2|d| - min(|d|,1))
    a = Bin(AluOp.ABSOLUTE_DIFF, Src0, Src1)
    m = minn(a, One)
    spec_q = Spec(
        body=((a + a) - m) * m,
        reference=lambda in0, in1, s0, s1, imm2: _np_sl1q(
            in0.reshape(in0.shape[0], -1).astype(np.float32)
            - in1.reshape(in1.shape[0], -1).astype(np.float32)),
    )

    def _acc_ref(fn):
        def ref(in0, in1, s0, s1, imm2):
            p = in0.shape[0]
            o = fn(in0.reshape(p, -1).astype(np.float32),
                   in1.reshape(p, -1).astype(np.float32) if in1 is not None
                   else None)
            init = np.asarray(s0).reshape(-1, 1) if isinstance(s0, np.ndarray) else s0
            return o, init + o.sum(axis=1, keepdims=True)
        return ref

    # MULR: out = in0*in1 ; accum = s0 + sum(out)
    spec_mulr = Spec(body=Src0 * Src1, accum=AluOp.ADD, accum_init=C0,
                     reference=_acc_ref(lambda a_, b_: a_ * b_))
    # ADDR: out = in0+in1 ; accum = s0 + sum(out)
    spec_addr = Spec(body=Src0 + Src1, accum=AluOp.ADD, accum_init=C0,
                     reference=_acc_ref(lambda a_, b_: a_ + b_))
    # NEGM: out = (1-in0)*in1 ; accum = s0 + sum(out)
    spec_negm = Spec(body=(One - Src0) * Src1, accum=AluOp.ADD, accum_init=C0,
                     reference=_acc_ref(lambda a_, b_: (1.0 - a_) * b_))

    ops = {}
    for name, spec in (("QSL1_ANT", spec_q), ("MULR_ANT", spec_mulr),
                       ("ADDR_ANT", spec_addr), ("NEGM_ANT", spec_negm)):
        if name in dve_ops._SUB_OPCODE_FOR_NAME:
            ops[name] = next(o for o in dve_ops.OPS if o.name == name)
            continue
        row = dve_ops._CUSTOM_DVE_ROW_BASE + len(dve_ops.OPS)
        shas = {}
        for ver in ("v3", "v4"):
            u = lower(spec, ver=ver)
            shas[ver] = DveOpSpec(name=name, opcode=row, uops=u,
                                  rd1_en=_has_src1(spec)).sha(ver)
        op = dve_ops.DveOp(name, spec, subdim=False, uops_sha=shas)
        dve_ops.OPS.append(op)
        dve_ops.CUSTOM_DVE_SPECS[name] = spec
        dve_ops._SUB_OPCODE_FOR_NAME[name] = row
        ops[name] = op
    return ops


def _install_act_root():
    """Restrict the ACT table universe to the one set holding every function
    we use (exp, ln, identity, copy), so walrus never ping-pongs table sets."""
    import os, json, shutil, tempfile
    if os.environ.get("BASS_ACT_ROOT_JSON_PATH"):
        return
    try:
        from neuronxcc.driver.Job import Job
        from neuronxcc.driver.jobs.support.FindActInfo import findActInfoFile
        src = findActInfoFile(Job.getPackageDir(), "gen3")
        d = json.load(open(src))
        keep = [t for t in d["act_func_sets"]
                if t["name"] == "natural_log_exp_and_others"]
        if not keep:
            return
        tmp = tempfile.mkdtemp(prefix="act_root_")
        srcdir = os.path.dirname(src)
        for t in keep:
            for k in d["pwp_file_keys"]:
                shutil.copy(os.path.join(srcdir, t[k]), tmp)
        with open(os.path.join(tmp, "act_info.json"), "w") as f:
            json.dump({"pwp_file_keys": d["pwp_file_keys"],
                       "act_func_sets": keep}, f)
        os.environ["BASS_ACT_ROOT_JSON_PATH"] = os.path.join(tmp, "act_info.json")
        # Make bass's pre-placed LoadActFuncSet ids consistent with the
        # stripped act_info: patch the table universe to the single set.
        import concourse.hw_specs as hw_specs
        _orig_gat = hw_specs.get_activation_tables

        def _gat(module_arch):
            full = _orig_gat(module_arch)
            return {"natural_log_exp_and_others":
                    full["natural_log_exp_and_others"]}

        hw_specs.get_activation_tables = _gat
        import concourse.bacc as _bacc_mod
        _bacc_mod.get_activation_tables = _gat
        import concourse.bass_interp as _bi_mod
        _bi_mod.get_activation_tables = _gat
    except Exception:
        pass


def build_bass():
    """Build the SPMD Bass module (one core's program)."""
    _install_act_root()
    ops = _register_custom_ops()
    nc = bacc.Bacc("TRN2")

    dram_in = {}
    dram_out = {}
    for lvl, H, W, nch in LEVELS:
        S = B_PER_CORE * H * W
        F = S // 128
        Fc = F // nch
        dram_in[f"cls{lvl}"] = nc.dram_tensor(
            f"cls{lvl}", [128, 4 * F], BF16, kind="ExternalInput")
        dram_in[f"gtm{lvl}"] = nc.dram_tensor(
            f"gtm{lvl}", [128, 3 * F], F32, kind="ExternalInput")
        dram_in[f"gtx{lvl}"] = nc.dram_tensor(
            f"gtx{lvl}", [nch, 128, 32 * Fc], BF16, kind="ExternalInput")
        dram_in[f"reg{lvl}"] = nc.dram_tensor(
            f"reg{lvl}", [nch, 128, 32 * Fc], BF16, kind="ExternalInput")
        dram_out[f"vn{lvl}"] = nc.dram_tensor(
            f"vn{lvl}", [128, F], F32, kind="ExternalOutput")
    dram_out["stats"] = nc.dram_tensor(
        "stats", [128, STATS_COLS], F32, kind="ExternalOutput")

    QSL1, MULR, ADDR, NEGM = (ops["QSL1_ANT"], ops["MULR_ANT"],
                              ops["ADDR_ANT"], ops["NEGM_ANT"])

    with tile.TileContext(nc) as tc:
        with (
            tc.tile_pool(name="io", bufs=4) as io,
            tc.tile_pool(name="lv", bufs=2) as lv,
            tc.tile_pool(name="wk", bufs=2) as wk,
            tc.tile_pool(name="st", bufs=1) as stp,
        ):
            stats = stp.tile([128, STATS_COLS], F32, name="stats_t")
            nc.gpsimd.memset(stats[:, :], 0.0)
            slot = 0
            for lvl, H, W, nch in LEVELS:
                S = B_PER_CORE * H * W
                F = S // 128
                Fc = F // nch
                base = slot * COLS_PER_CHUNK
                regcols = [(slot + j) * COLS_PER_CHUNK for j in range(nch)]
                slot += nch
                col = lambda i: stats[:, base + i:base + i + 1]

                # per-chunk reg/map tiles + DMAs (first chunk's loads lead)
                REGH = [[io.tile([128, 16 * Fc], BF16, tag=f"reg{h}",
                                 name=f"reg_{lvl}_{j}_{h}") for h in range(2)]
                        for j in range(nch)]
                GTXH = [[io.tile([128, 16 * Fc], BF16, tag=f"gtx{h}",
                                 name=f"gtx_{lvl}_{j}_{h}") for h in range(2)]
                        for j in range(nch)]
                for h in range(2):
                    nc.sync.dma_start(
                        REGH[0][h][:, :],
                        dram_in[f"reg{lvl}"][0][:, 16 * h * Fc:16 * (h + 1) * Fc])
                    nc.sync.dma_start(
                        GTXH[0][h][:, :],
                        dram_in[f"gtx{lvl}"][0][:, 16 * h * Fc:16 * (h + 1) * Fc])

                GTM = lv.tile([128, 3 * F], F32, tag="gtmL", name=f"gtm_{lvl}")
                CLS = lv.tile([128, 4 * F], BF16, tag="clsL", name=f"cls_{lvl}")
                nc.sync.dma_start(GTM[:, :], dram_in[f"gtm{lvl}"][:, :])
                nc.sync.dma_start(CLS[:, :], dram_in[f"cls{lvl}"][:, :])
                for j in range(1, nch):
                    for h in range(2):
                        nc.sync.dma_start(
                            REGH[j][h][:, :],
                            dram_in[f"reg{lvl}"][j][:, 16 * h * Fc:16 * (h + 1) * Fc])
                        nc.sync.dma_start(
                            GTXH[j][h][:, :],
                            dram_in[f"gtx{lvl}"][j][:, 16 * h * Fc:16 * (h + 1) * Fc])

                tr = GTM[:, 0:F]
                tcl = GTM[:, F:2 * F]
                train = GTM[:, 2 * F:3 * F]

                # --- masks (whole level at once) ---
                pos = lv.tile([128, F], F32, tag="posL", name=f"pos_{lvl}")
                neg = lv.tile([128, F], F32, tag="negL", name=f"neg_{lvl}")
                w2 = lv.tile([128, F], F32, tag="w2L", name=f"w2_{lvl}")
                nc.vector._custom_dve(MULR, out=pos[:, :], in0=tr, in1=train,
                                      s0=0.0, accum_out=col(C_NPOS))
                nc.vector._custom_dve(NEGM, out=neg[:, :], in0=tr, in1=train,
                                      s0=0.0, accum_out=col(C_NEGCNT))
                # w2 = (1 + tcl) * pos  ==  (tr + tcl) * pos  for 0/1 masks
                nc.vector.scalar_tensor_tensor(
                    out=w2[:, :], in0=tcl, scalar=1.0, in1=pos[:, :],
                    op0=ALU.add, op1=ALU.mult)
                w2h = lv.tile([128, F], BF16, tag="w2hL", name=f"w2h_{lvl}")
                nc.scalar.copy(w2h[:, :], w2[:, :])

                # --- CE head: d = (1-2t)*(hi-lo) for tr and tcl at once ---
                sgn = lv.tile([128, 2 * F], BF16, tag="sgnL", name=f"sgn_{lvl}")
                diff = lv.tile([128, 2 * F], BF16, tag="diffL", name=f"diff_{lvl}")
                dce = lv.tile([128, 2 * F], BF16, tag="dceL", name=f"dce_{lvl}")
                nc.scalar.activation(sgn[:, :], GTM[:, 0:2 * F],
                                     ACT.Identity, bias=1.0, scale=-2.0)
                cls3d = CLS[:, :].rearrange("p (g f) -> p g f", g=2)
                nc.vector.tensor_tensor(
                    out=diff[:, :].rearrange("p (g f) -> p g f", g=2),
                    in0=cls3d[:, :, F:2 * F], in1=cls3d[:, :, 0:F],
                    op=ALU.subtract)
                nc.vector.tensor_mul(dce[:, :], diff[:, :], sgn[:, :])

                expd = lv.tile([128, 2 * F], F32, tag="expdL", name=f"expd_{lvl}")
                ce = lv.tile([128, 2 * F], F32, tag="ceL", name=f"ce_{lvl}")
                nc.scalar.activation(expd[:, :], dce[:, :], ACT.Exp)
                nc.scalar.activation(ce[:, 0:F], expd[:, 0:F], ACT.Ln, bias=1.0)
                nc.scalar.activation(ce[:, F:2 * F], expd[:, F:2 * F],
                                     ACT.Ln, bias=1.0, accum_out=col(C_TCLALL))

                # --- CE stats ---
                ce_sc = lv.tile([128, 2 * F], F32, tag="cescL", bufs=1,
                                name=f"cesc_{lvl}")
                nc.vector._custom_dve(
                    MULR, out=ce_sc[:, 0:F], in0=pos[:, :], in1=ce[:, 0:F],
                    s0=0.0, accum_out=col(C_LOSSPOS))
                nc.vector._custom_dve(
                    MULR, out=ce_sc[:, F:2 * F], in0=pos[:, :],
                    in1=ce[:, F:2 * F], s0=0.0, accum_out=col(C_TCLPOS))

                # --- masked negatives out: vn = (ce_tr + 1) * neg ---
                vn = lv.tile([128, F], F32, tag="vnL", name=f"vn_{lvl}")
                nc.vector.scalar_tensor_tensor(
                    out=vn[:, :], in0=ce[:, 0:F], scalar=1.0, in1=neg[:, :],
                    op0=ALU.add, op1=ALU.mult)
                nc.scalar.dma_start(dram_out[f"vn{lvl}"][:, :], vn[:, :])

                # --- regression smooth-L1 per chunk, x then y halves ---
                for j in range(nch):
                    w2b = w2h[:, j * Fc:(j + 1) * Fc].unsqueeze(1) \
                        .to_broadcast((128, 16, Fc))
                    for half, ccol in ((0, C_REGX), (1, C_REGY)):
                        q = wk.tile([128, 16 * Fc], BF16, tag="q",
                                    name=f"q_{lvl}_{j}_{half}")
                        qsc = wk.tile([128, 16 * Fc], BF16, tag="qsc", bufs=1,
                                      name=f"qsc_{lvl}_{j}_{half}")
                        nc.vector._custom_dve(
                            QSL1,
                            out=q[:, :].rearrange("p (k f) -> p k f", k=16),
                            in0=GTXH[j][half][:, :]
                                .rearrange("p (k f) -> p k f", k=16),
                            in1=REGH[j][half][:, :]
                                .rearrange("p (k f) -> p k f", k=16))
                        nc.vector._custom_dve(
                            MULR,
                            out=qsc[:, :].rearrange("p (k f) -> p k f", k=16),
                            in0=q[:, :].rearrange("p (k f) -> p k f", k=16),
                            in1=w2b, s0=0.0,
                            accum_out=stats[:, regcols[j] + ccol:
                                            regcols[j] + ccol + 1])

            nc.scalar.dma_start(dram_out["stats"][:, :], stats[:, :])

    nc.compile()
    return nc


def prep_core_inputs(inputs, core):
    """Shard + relayout one core's inputs: [2,C,H,W] -> [nch,128,C*Fc]."""
    b0 = core * B_PER_CORE
    out = {}
    for lvl, H, W, nch in LEVELS:
        S = B_PER_CORE * H * W
        F = S // 128
        Fc = F // nch
        def relayout(X, dtype, n):
            C = X.shape[1]
            Y = X.transpose(1, 0, 2, 3).reshape(C, 128, n, F // n)
            r = Y.transpose(2, 1, 0, 3).reshape(n, 128, C * (F // n))
            return np.ascontiguousarray(r[0] if n == 1 else r).astype(dtype)

        cls = inputs[f"cls{lvl}"][b0:b0 + B_PER_CORE]
        gt = inputs[f"gt{lvl}"][b0:b0 + B_PER_CORE]
        reg = inputs[f"reg{lvl}"][b0:b0 + B_PER_CORE]
        out[f"cls{lvl}"] = relayout(cls, NP_BF16, 1)
        out[f"gtm{lvl}"] = relayout(gt[:, 0:3], np.float32, 1)
        out[f"gtx{lvl}"] = relayout(gt[:, 3:35], NP_BF16, nch)
        out[f"reg{lvl}"] = relayout(reg, NP_BF16, nch)
    return out


def finish_host(results):
    """Merge per-core device partials into the final [4] loss vector."""
    total = np.zeros(4, dtype=np.float64)
    for li, (lvl, H, W, nch) in enumerate(LEVELS):
        slot0 = sum(n for _, _, _, n in LEVELS[:li])
        n_pos = neg_cnt = loss_pos = tcl_pos = tcl_all = accx = accy = 0.0
        neg_vals = []
        for r in results:
            st = np.asarray(r["stats"], dtype=np.float64)
            for t in range(slot0, slot0 + nch):
                b = t * COLS_PER_CHUNK
                n_pos += st[:, b + C_NPOS].sum()
                neg_cnt += st[:, b + C_NEGCNT].sum()
                loss_pos += st[:, b + C_LOSSPOS].sum()
                tcl_pos += st[:, b + C_TCLPOS].sum()
                tcl_all += st[:, b + C_TCLALL].sum()
                accx += st[:, b + C_REGX].sum()
                accy += st[:, b + C_REGY].sum()
            v = np.asarray(r[f"vn{lvl}"], dtype=np.float32).ravel()
            neg_vals.append(v[v > 0.0] - 1.0)
        neg_vals = np.concatenate(neg_vals) if neg_vals else np.zeros(0, np.float32)

        M = 16 * H * W
        n_pos_i = int(round(n_pos))
        neg_cnt_i = int(round(neg_cnt))
        if n_pos_i > 0:
            n_neg = min(neg_cnt_i,
                        int(np.floor(np.float32(OHEM_RATIO) * np.float32(n_pos_i))))
        else:
            n_neg = 100
        k = min(n_neg, neg_vals.size)
        if k > 0:
            loss_neg = float(np.partition(neg_vals, neg_vals.size - k)
                             [neg_vals.size - k:].astype(np.float64).sum())
        else:
            loss_neg = 0.0
        loss_tr = (loss_pos + loss_neg) / (n_pos_i + float(n_neg))

        if n_pos_i > 0:
            mean_pos = tcl_pos / max(n_pos_i, 1)
            mean_neg = (tcl_all - tcl_pos) / max(M - n_pos_i, 1)
            loss_tcl = mean_pos + 0.5 * mean_neg
            denom = max(n_pos_i, 1) * KCH
            loss_rx = 0.25 * accx / denom
            loss_ry = 0.25 * accy / denom
        else:
            loss_tcl = loss_rx = loss_ry = 0.0
        total += np.array([loss_tr, loss_tcl, loss_rx, loss_ry])
    return total.astype(np.float32)


_NC_CACHE = None


def _get_nc():
    global _NC_CACHE
    if _NC_CACHE is None:
        _NC_CACHE = build_bass()
    return _NC_CACHE


def run_device(in_maps, trace=False):
    from concourse.bass_utils import run_bass_kernel_spmd
    nc = _get_nc()
    return run_bass_kernel_spmd(nc, in_maps, list(range(NCORES)), trace=trace)


def kernel(**inputs) -> np.ndarray:
    in_maps = [prep_core_inputs(inputs, c) for c in range(NCORES)]
    res = run_device(in_maps)
    return finish_host(res.results)


# revision 19
# speedup vs baseline: 1.0257x; 1.0257x over previous
"""Trainium2 Bass kernel for nn_BSLoss (text-snake style OHEM loss), 8-core
data-parallel.

Strategy
--------
Host shards the batch dim (16 -> 2 per core) and re-lays every tensor out as
[nchunks, 128, C*Fc] fp32 blocks (partition-blocked spatial, channel blocks
adjacent in the free dim) so each chunk is one contiguous DMA and every
channel block of every tensor shares the same [128, Fc] spatial layout.

Device (per core, identical SPMD program):
  - masks: pos = tr*train (+count), neg = (1-tr)*train (+count), w2=(tr+tcl)*pos
  - 2-class CE for tr and tcl heads: ce = relu(d) + ln(1+exp(-|d|)),
    d = (1-2t)*(logit1-logit0)  [== -log_softmax picked]
  - weighted smooth-L1 over the 32 regression channels via two custom DVE
    ops: q = min(|xm-xp|,1)*(2|xm-xp|-min(..,1)) == 2*smooth_l1, then
    accum += sum(w2 * q) with w2 broadcast across channel blocks
  - per-chunk partial sums land in a [128, NCOLS] stats tile; the masked
    negative-CE values (for global topk OHEM) are written back to HBM
Host merges: sums partials, does the exact global top-k over ~2MB of masked
CE values, and finishes the scalar divisions (exactly mirroring reference
semantics, incl. the n_pos==0 fallbacks).
"""

import numpy as np
import ml_dtypes

import concourse.bacc as bacc
import concourse.mybir as mybir
import concourse.dve_ops as dve_ops
from concourse.dve_spec import (
    Spec, Src0, Src1, C0, Zero, One, AluOp, Bin, minn, lower, _has_src1,
)
from concourse.dve_uop import DveOpSpec
from concourse import tile

F32 = mybir.dt.float32
BF16 = mybir.dt.bfloat16
NP_BF16 = ml_dtypes.bfloat16
ALU = mybir.AluOpType
ACT = mybir.ActivationFunctionType

NCORES = 8
B_PER_CORE = 2
# level -> (H, W, nchunks)
LEVELS = [(3, 160, 160, 4), (4, 80, 80, 1), (5, 40, 40, 1)]
KCH = 16          # regression channels per axis
OHEM_RATIO = 3.0

# stats tile column layout: per chunk-slot t, base = t*12
C_NPOS, C_NEGCNT, C_LOSSPOS, C_TCLPOS, C_TCLALL, C_REGX, C_REGY = range(7)
C_DUM0, C_DUM1, C_DUM2, C_DUM3, C_DUM4 = range(7, 12)
COLS_PER_CHUNK = 12
N_CHUNK_SLOTS = sum(nch for _, _, _, nch in LEVELS)
STATS_COLS = COLS_PER_CHUNK * N_CHUNK_SLOTS


def _np_sl1q(d):
    a = np.abs(d)
    m = np.minimum(a, 1.0)
    return m * (a + a - m)   # == 2 * smooth_l1(d)


def _register_custom_ops():
    """Register our fused DVE ops (idempotent)."""
    # QSL1: out = q(Src0 - Src1), q(d) = min(|d|,1)*(2|d| - min(|d|,1))
    a = Bin(AluOp.ABSOLUTE_DIFF, Src0, Src1)
    m = minn(a, One)
    spec_q = Spec(
        body=((a + a) - m) * m,
        reference=lambda in0, in1, s0, s1, imm2: _np_sl1q(
            in0.reshape(in0.shape[0], -1).astype(np.float32)
            - in1.reshape(in1.shape[0], -1).astype(np.float32)),
    )

    def _acc_ref(fn):
        def ref(in0, in1, s0, s1, imm2):
            p = in0.shape[0]
            o = fn(in0.reshape(p, -1).astype(np.float32),
                   in1.reshape(p, -1).astype(np.float32) if in1 is not None
                   else None)
            init = np.asarray(s0).reshape(-1, 1) if isinstance(s0, np.ndarray) else s0
            return o, init + o.sum(axis=1, keepdims=True)
        return ref

    # MULR: out = in0*in1 ; accum = s0 + sum(out)
    spec_mulr = Spec(body=Src0 * Src1, accum=AluOp.ADD, accum_init=C0,
                     reference=_acc_ref(lambda a_, b_: a_ * b_))
    # ADDR: out = in0+in1 ; accum = s0 + sum(out)
    spec_addr = Spec(body=Src0 + Src1, accum=AluOp.ADD, accum_init=C0,
                     reference=_acc_ref(lambda a_, b_: a_ + b_))
    # NEGM: out = (1-in0)*in1 ; accum = s0 + sum(out)
    spec_negm = Spec(body=(One - Src0) * Src1, accum=AluOp.ADD, accum_init=C0,
                     reference=_acc_ref(lambda a_, b_: (1.0 - a_) * b_))

    ops = {}
    for name, spec in (("QSL1_ANT", spec_q), ("MULR_ANT", spec_mulr),
                       ("ADDR_ANT", spec_addr), ("NEGM_ANT", spec_negm)):
        if name in dve_ops._SUB_OPCODE_FOR_NAME:
            ops[name] = next(o for o in dve_ops.OPS if o.name == name)
            continue
        row = dve_ops._CUSTOM_DVE_ROW_BASE + len(dve_ops.OPS)
        shas = {}
        for ver in ("v3", "v4"):
            u = lower(spec, ver=ver)
            shas[ver] = DveOpSpec(name=name, opcode=row, uops=u,
                                  rd1_en=_has_src1(spec)).sha(ver)
        op = dve_ops.DveOp(name, spec, subdim=False, uops_sha=shas)
        dve_ops.OPS.append(op)
        dve_ops.CUSTOM_DVE_SPECS[name] = spec
        dve_ops._SUB_OPCODE_FOR_NAME[name] = row
        ops[name] = op
    return ops


def _install_act_root():
    """Restrict the ACT table universe to the one set holding every function
    we use (exp, ln, identity, copy), so walrus never ping-pongs table sets."""
    import os, json, shutil, tempfile
    if os.environ.get("BASS_ACT_ROOT_JSON_PATH"):
        return
    try:
        from neuronxcc.driver.Job import Job
        from neuronxcc.driver.jobs.support.FindActInfo import findActInfoFile
        src = findActInfoFile(Job.getPackageDir(), "gen3")
        d = json.load(open(src))
        keep = [t for t in d["act_func_sets"]
                if t["name"] == "natural_log_exp_and_others"]
        if not keep:
            return
        tmp = tempfile.mkdtemp(prefix="act_root_")
        srcdir = os.path.dirname(src)
        for t in keep:
            for k in d["pwp_file_keys"]:
                shutil.copy(os.path.join(srcdir, t[k]), tmp)
        with open(os.path.join(tmp, "act_info.json"), "w") as f:
            json.dump({"pwp_file_keys": d["pwp_file_keys"],
                       "act_func_sets": keep}, f)
        os.environ["BASS_ACT_ROOT_JSON_PATH"] = os.path.join(tmp, "act_info.json")
        # Make bass's pre-placed LoadActFuncSet ids consistent with the
        # stripped act_info: patch the table universe to the single set.
        import concourse.hw_specs as hw_specs
        _orig_gat = hw_specs.get_activation_tables

        def _gat(module_arch):
            full = _orig_gat(module_arch)
            return {"natural_log_exp_and_others":
                    full["natural_log_exp_and_others"]}

        hw_specs.get_activation_tables = _gat
        import concourse.bacc as _bacc_mod
        _bacc_mod.get_activation_tables = _gat
        import concourse.bass_interp as _bi_mod
        _bi_mod.get_activation_tables = _gat
    except Exception:
        pass


def build_bass():
    """Build the SPMD Bass module (one core's program)."""
    _install_act_root()
    ops = _register_custom_ops()
    nc = bacc.Bacc("TRN2")

    dram_in = {}
    dram_out = {}
    for lvl, H, W, nch in LEVELS:
        S = B_PER_CORE * H * W
        F = S // 128
        Fc = F // nch
        dram_in[f"cls{lvl}"] = nc.dram_tensor(
            f"cls{lvl}", [128, 4 * F], BF16, kind="ExternalInput")
        dram_in[f"gtm{lvl}"] = nc.dram_tensor(
            f"gtm{lvl}", [128, 3 * F], F32, kind="ExternalInput")
        dram_in[f"gtx{lvl}"] = nc.dram_tensor(
            f"gtx{lvl}", [nch, 128, 32 * Fc], BF16, kind="ExternalInput")
        dram_in[f"reg{lvl}"] = nc.dram_tensor(
            f"reg{lvl}", [nch, 128, 32 * Fc], BF16, kind="ExternalInput")
        dram_out[f"vn{lvl}"] = nc.dram_tensor(
            f"vn{lvl}", [128, F], F32, kind="ExternalOutput")
    dram_out["stats"] = nc.dram_tensor(
        "stats", [128, STATS_COLS], F32, kind="ExternalOutput")

    QSL1, MULR, ADDR, NEGM = (ops["QSL1_ANT"], ops["MULR_ANT"],
                              ops["ADDR_ANT"], ops["NEGM_ANT"])

    with tile.TileContext(nc) as tc:
        with (
            tc.tile_pool(name="io", bufs=4) as io,
            tc.tile_pool(name="lv", bufs=2) as lv,
            tc.tile_pool(name="wk", bufs=2) as wk,
            tc.tile_pool(name="st", bufs=1) as stp,
        ):
            stats = stp.tile([128, STATS_COLS], F32, name="stats_t")
            nc.gpsimd.memset(stats[:, :], 0.0)
            slot = 0
            for lvl, H, W, nch in LEVELS:
                S = B_PER_CORE * H * W
                F = S // 128
                Fc = F // nch
                base = slot * COLS_PER_CHUNK
                regcols = [(slot + j) * COLS_PER_CHUNK for j in range(nch)]
                slot += nch
                col = lambda i: stats[:, base + i:base + i + 1]

                # per-chunk reg/map tiles + DMAs (first chunk's loads lead)
                REGH = [[io.tile([128, 16 * Fc], BF16, tag=f"reg{h}",
                                 name=f"reg_{lvl}_{j}_{h}") for h in range(2)]
                        for j in range(nch)]
                GTXH = [[io.tile([128, 16 * Fc], BF16, tag=f"gtx{h}",
                                 name=f"gtx_{lvl}_{j}_{h}") for h in range(2)]
                        for j in range(nch)]
                for h in range(2):
                    nc.sync.dma_start(
                        REGH[0][h][:, :],
                        dram_in[f"reg{lvl}"][0][:, 16 * h * Fc:16 * (h + 1) * Fc])
                    nc.sync.dma_start(
                        GTXH[0][h][:, :],
                        dram_in[f"gtx{lvl}"][0][:, 16 * h * Fc:16 * (h + 1) * Fc])

                GTM = lv.tile([128, 3 * F], F32, tag="gtmL", name=f"gtm_{lvl}")
                CLS = lv.tile([128, 4 * F], BF16, tag="clsL", name=f"cls_{lvl}")
                nc.sync.dma_start(GTM[:, :], dram_in[f"gtm{lvl}"][:, :])
                nc.sync.dma_start(CLS[:, :], dram_in[f"cls{lvl}"][:, :])
                for j in range(1, nch):
                    for h in range(2):
                        nc.sync.dma_start(
                            REGH[j][h][:, :],
                            dram_in[f"reg{lvl}"][j][:, 16 * h * Fc:16 * (h + 1) * Fc])
                        nc.sync.dma_start(
                            GTXH[j][h][:, :],
                            dram_in[f"gtx{lvl}"][j][:, 16 * h * Fc:16 * (h + 1) * Fc])

                tr = GTM[:, 0:F]
                tcl = GTM[:, F:2 * F]
                train = GTM[:, 2 * F:3 * F]

                # --- masks (whole level at once) ---
                pos = lv.tile([128, F], F32, tag="posL", name=f"pos_{lvl}")
                neg = lv.tile([128, F], F32, tag="negL", name=f"neg_{lvl}")
                w2 = lv.tile([128, F], F32, tag="w2L", name=f"w2_{lvl}")
                nc.vector._custom_dve(MULR, out=pos[:, :], in0=tr, in1=train,
                                      s0=0.0, accum_out=col(C_NPOS))
                nc.vector._custom_dve(NEGM, out=neg[:, :], in0=tr, in1=train,
                                      s0=0.0, accum_out=col(C_NEGCNT))
                # w2 = (1 + tcl) * pos  ==  (tr + tcl) * pos  for 0/1 masks
                nc.vector.scalar_tensor_tensor(
                    out=w2[:, :], in0=tcl, scalar=1.0, in1=pos[:, :],
                    op0=ALU.add, op1=ALU.mult)
                w2h = lv.tile([128, F], BF16, tag="w2hL", name=f"w2h_{lvl}")
                nc.scalar.copy(w2h[:, :], w2[:, :])

                # --- CE head: d = (1-2t)*(hi-lo) for tr and tcl at once ---
                sgn = lv.tile([128, 2 * F], BF16, tag="sgnL", name=f"sgn_{lvl}")
                diff = lv.tile([128, 2 * F], BF16, tag="diffL", name=f"diff_{lvl}")
                dce = lv.tile([128, 2 * F], BF16, tag="dceL", name=f"dce_{lvl}")
                nc.scalar.activation(sgn[:, :], GTM[:, 0:2 * F],
                                     ACT.Identity, bias=1.0, scale=-2.0)
                cls3d = CLS[:, :].rearrange("p (g f) -> p g f", g=2)
                nc.vector.tensor_tensor(
                    out=diff[:, :].rearrange("p (g f) -> p g f", g=2),
                    in0=cls3d[:, :, F:2 * F], in1=cls3d[:, :, 0:F],
                    op=ALU.subtract)
                nc.vector.tensor_mul(dce[:, :], diff[:, :], sgn[:, :])

                expd = lv.tile([128, 2 * F], F32, tag="expdL", name=f"expd_{lvl}")
                ce = lv.tile([128, 2 * F], F32, tag="ceL", name=f"ce_{lvl}")
                nc.scalar.activation(expd[:, :], dce[:, :], ACT.Exp)
                nc.scalar.activation(ce[:, 0:F], expd[:, 0:F], ACT.Ln, bias=1.0)
                nc.scalar.activation(ce[:, F:2 * F], expd[:, F:2 * F],
                                     ACT.Ln, bias=1.0, accum_out=col(C_TCLALL))

                # --- CE stats ---
                ce_sc = lv.tile([128, 2 * F], F32, tag="cescL", bufs=1,
                                name=f"cesc_{lvl}")
                nc.vector._custom_dve(
                    MULR, out=ce_sc[:, 0:F], in0=pos[:, :], in1=ce[:, 0:F],
                    s0=0.0, accum_out=col(C_LOSSPOS))
                nc.vector._custom_dve(
                    MULR, out=ce_sc[:, F:2 * F], in0=pos[:, :],
                    in1=ce[:, F:2 * F], s0=0.0, accum_out=col(C_TCLPOS))

                # --- masked negatives out: vn = (ce_tr + 1) * neg ---
                vn = lv.tile([128, F], F32, tag="vnL", name=f"vn_{lvl}")
                nc.vector.scalar_tensor_tensor(
                    out=vn[:, :], in0=ce[:, 0:F], scalar=1.0, in1=neg[:, :],
                    op0=ALU.add, op1=ALU.mult)
                nc.scalar.dma_start(dram_out[f"vn{lvl}"][:, :], vn[:, :])

                # --- regression smooth-L1 per chunk, x then y halves ---
                for j in range(nch):
                    w2b = w2h[:, j * Fc:(j + 1) * Fc].unsqueeze(1) \
                        .to_broadcast((128, 16, Fc))
                    for half, ccol in ((0, C_REGX), (1, C_REGY)):
                        q = wk.tile([128, 16 * Fc], BF16, tag="q",
                                    name=f"q_{lvl}_{j}_{half}")
                        qsc = wk.tile([128, 16 * Fc], BF16, tag="qsc", bufs=1,
                                      name=f"qsc_{lvl}_{j}_{half}")
                        nc.vector._custom_dve(
                            QSL1,
                            out=q[:, :].rearrange("p (k f) -> p k f", k=16),
                            in0=GTXH[j][half][:, :]
                                .rearrange("p (k f) -> p k f", k=16),
                            in1=REGH[j][half][:, :]
                                .rearrange("p (k f) -> p k f", k=16))
                        nc.vector._custom_dve(
                            MULR,
                            out=qsc[:, :].rearrange("p (k f) -> p k f", k=16),
                            in0=q[:, :].rearrange("p (k f) -> p k f", k=16),
                            in1=w2b, s0=0.0,
                            accum_out=stats[:, regcols[j] + ccol:
                                            regcols[j] + ccol + 1])

            nc.scalar.dma_start(dram_out["stats"][:, :], stats[:, :])

    nc.compile()
    return nc


def prep_core_inputs(inputs, core):
    """Shard + relayout one core's inputs: [2,C,H,W] -> [nch,128,C*Fc]."""
    b0 = core * B_PER_CORE
    out = {}
    for lvl, H, W, nch in LEVELS:
        S = B_PER_CORE * H * W
        F = S // 128
        Fc = F // nch
        def relayout(X, dtype, n):
            C = X.shape[1]
            Y = X.transpose(1, 0, 2, 3).reshape(C, 128, n, F // n)
            r = Y.transpose(2, 1, 0, 3).reshape(n, 128, C * (F // n))
            return np.ascontiguousarray(r[0] if n == 1 else r).astype(dtype)

        cls = inputs[f"cls{lvl}"][b0:b0 + B_PER_CORE]
        gt = inputs[f"gt{lvl}"][b0:b0 + B_PER_CORE]
        reg = inputs[f"reg{lvl}"][b0:b0 + B_PER_CORE]
        out[f"cls{lvl}"] = relayout(cls, NP_BF16, 1)
        out[f"gtm{lvl}"] = relayout(gt[:, 0:3], np.float32, 1)
        out[f"gtx{lvl}"] = relayout(gt[:, 3:35], NP_BF16, nch)
        out[f"reg{lvl}"] = relayout(reg, NP_BF16, nch)
    return out


def finish_host(results):
    """Merge per-core device partials into the final [4] loss vector."""
    total = np.zeros(4, dtype=np.float64)
    for li, (lvl, H, W, nch) in enumerate(LEVELS):
        slot0 = sum(n for _, _, _, n in LEVELS[:li])
        n_pos = neg_cnt = loss_pos = tcl_pos = tcl_all = accx = accy = 0.0
        neg_vals = []
        for r in results:
            st = np.asarray(r["stats"], dtype=np.float64)
            for t in range(slot0, slot0 + nch):
                b = t * COLS_PER_CHUNK
                n_pos += st[:, b + C_NPOS].sum()
                neg_cnt += st[:, b + C_NEGCNT].sum()
                loss_pos += st[:, b + C_LOSSPOS].sum()
                tcl_pos += st[:, b + C_TCLPOS].sum()
                tcl_all += st[:, b + C_TCLALL].sum()
                accx += st[:, b + C_REGX].sum()
                accy += st[:, b + C_REGY].sum()
            v = np.asarray(r[f"vn{lvl}"], dtype=np.float32).ravel()
            neg_vals.append(v[v > 0.0] - 1.0)
        neg_vals = np.concatenate(neg_vals) if neg_vals else np.zeros(0, np.float32)

        M = 16 * H * W
        n_pos_i = int(round(n_pos))
        neg_cnt_i = int(round(neg_cnt))
        if n_pos_i > 0:
            n_neg = min(neg_cnt_i,
                        int(np.floor(np.float32(OHEM_RATIO) * np.float32(n_pos_i))))
        else:
            n_neg = 100
        k = min(n_neg, neg_vals.size)
        if k > 0:
            loss_neg = float(np.partition(neg_vals, neg_vals.size - k)
                             [neg_vals.size - k:].astype(np.float64).sum())
        else:
            loss_neg = 0.0
        loss_tr = (loss_pos + loss_neg) / (n_pos_i + float(n_neg))

        if n_pos_i > 0:
            mean_pos = tcl_pos / max(n_pos_i, 1)
            mean_neg = (tcl_all - tcl_pos) / max(M - n_pos_i, 1)
            loss_tcl = mean_pos + 0.5 * mean_neg
            denom = max(n_pos_i, 1) * KCH
            loss_rx = 0.25 * accx / denom
            loss_ry = 0.25 * accy / denom
        else:
            loss_tcl = loss_rx = loss_ry = 0.0
        total += np.array([loss_tr, loss_tcl, loss_rx, loss_ry])
    return total.astype(np.float32)


_NC_CACHE = None


def _get_nc():
    global _NC_CACHE
    if _NC_CACHE is None:
        _NC_CACHE = build_bass()
    return _NC_CACHE


def run_device(in_maps, trace=False):
    from concourse.bass_utils import run_bass_kernel_spmd
    nc = _get_nc()
    return run_bass_kernel_spmd(nc, in_maps, list(range(NCORES)), trace=trace)


def kernel(**inputs) -> np.ndarray:
    in_maps = [prep_core_inputs(inputs, c) for c in range(NCORES)]
    res = run_device(in_maps)
    return finish_host(res.results)
